# revision 1
# baseline (speedup 1.0000x reference)
"""RBF kernel ridge regression inference on 8 Trainium2 NeuronCores.

out[q] = sum_t exp(-gamma * ||X[q] - T[t]||^2) * coef[t]

Factored as exp(-g*x2[q]) * sum_t exp(2g*dot[t,q] - g*y2[t]) * coef[t] so the
whole inner computation maps onto TensorE (bf16 GEMM + matvec) and ScalarE
(one fused exp with per-partition bias).  Row norms are computed on DVE
(square+accumulate in one scalar_tensor_tensor op) so ScalarE runs Exp only
and never reloads its activation table.  Queries are sharded across the 8
cores; train_X and dual_coef are replicated.
"""

import numpy as np
import ml_dtypes

GAMMA = 1.0
N_QUERY, N_TRAIN, D = 8192, 8192, 512
N_CORES = 8
P = 128
QPC = N_QUERY // N_CORES  # 1024 queries per core
ND = D // P               # 4 contraction chunks
NT = N_TRAIN // P         # 64 train chunks
TGROUP = 8                # train chunks per resident tt DMA group
NTG = NT // TGROUP
QTILE = 512               # free dim of one sq-dist psum tile
NQC = QPC // QTILE        # 2 query chunks per core
NXC = QPC // P            # 8 query columns for x2 accumulation

_CACHE = {}


def _build_program(repeats=1):
    from contextlib import ExitStack

    import concourse.bass as bass
    import concourse.mybir as mybir
    import concourse.tile as tile
    from concourse import bacc

    f32 = mybir.dt.float32
    bf16 = mybir.dt.bfloat16
    AF = mybir.ActivationFunctionType
    MUL = mybir.AluOpType.mult

    nc = bacc.Bacc(
        "TRN2", target_bir_lowering=False, debug=False, num_devices=N_CORES
    )

    tt_d = nc.dram_tensor("tt_bf16", [D, N_TRAIN], bf16, kind="ExternalInput").ap()
    xt_d = nc.dram_tensor("xt_bf16", [D, QPC], bf16, kind="ExternalInput").ap()
    trf_d = nc.dram_tensor("train_f32", [N_TRAIN, D], f32, kind="ExternalInput").ap()
    xf_d = nc.dram_tensor("x_f32", [QPC, D], f32, kind="ExternalInput").ap()
    coef_d = nc.dram_tensor("coef_bf16", [P, NT], bf16, kind="ExternalInput").ap()
    out_d = nc.dram_tensor("out", [QPC], f32, kind="ExternalOutput").ap()
    x2r_d = nc.dram_tensor("x2_bounce", [QPC], f32).ap()  # internal scratch

    with tile.TileContext(nc) as tc, ExitStack() as ctx:
        res = ctx.enter_context(tc.tile_pool(name="res", bufs=1))
        ttp = ctx.enter_context(tc.tile_pool(name="ttp", bufs=1))
        stream = ctx.enter_context(tc.tile_pool(name="stream", bufs=4))
        exppool = ctx.enter_context(tc.tile_pool(name="expp", bufs=4))
        sqpool = ctx.enter_context(tc.tile_pool(name="psq", bufs=4, space="PSUM"))
        spool = ctx.enter_context(tc.tile_pool(name="pS", bufs=1, space="PSUM"))

        # ---- prologue: resident loads + x2 = rowwise ||X||^2 ----
        xt_sb = []
        for dc in range(ND):
            t = res.tile([P, QPC], bf16, tag=f"xt{dc}")
            nc.sync.dma_start(t[:], xt_d[dc * P : (dc + 1) * P, :])
            xt_sb.append(t)
        coef_sb = res.tile([P, NT], bf16, tag="coef")
        nc.sync.dma_start(coef_sb[:], coef_d[:])

        # x2 in column layout via DVE square+accumulate, then bounce through
        # DRAM to transpose into a single [1, QPC] row (hidden under main loop)
        x2_sb = res.tile([P, NXC], f32, tag="x2")
        for c in range(NXC):
            xtile = stream.tile([P, D], f32, tag="xf")
            nc.sync.dma_start(xtile[:], xf_d[c * P : (c + 1) * P, :])
            scr = stream.tile([P, D], bf16, tag="xscr")
            nc.vector.scalar_tensor_tensor(
                scr[:], xtile[:], 1.0, xtile[:], MUL, MUL,
                accum_out=x2_sb[:, c : c + 1],
            )
        nc.sync.dma_start(x2r_d.rearrange("(c p) -> p c", p=P), x2_sb[:])
        x2row = res.tile([1, QPC], f32, tag="x2row")
        nc.sync.dma_start(x2row[:], x2r_d.rearrange("(a q) -> a q", a=1))

        # ---- main loop over train chunks ----
        # S[qc] accumulates sum_t exp(...) * coef[t] as a [1, 512] psum row
        # per query chunk; each lives in its own psum bank so the long
        # accumulation groups never share a zero region.
        S_ps = [
            spool.tile([1, QTILE], f32, tag=f"S{qc}", name=f"S{qc}")
            for qc in range(NQC)
        ]
        for g in range(NTG):
            ttg = []
            for dc in range(ND):
                t = ttp.tile([P, TGROUP * P], bf16, tag=f"tt_{dc}_{g}")
                nc.sync.dma_start(
                    t[:],
                    tt_d[dc * P : (dc + 1) * P, g * TGROUP * P : (g + 1) * TGROUP * P],
                )
                ttg.append(t)
            for tl in range(TGROUP):
                ti = g * TGROUP + tl
                # y2n = -gamma * ||T[t]||^2 for this chunk (DVE, one op)
                trt = stream.tile([P, D], f32, tag="trf")
                nc.sync.dma_start(trt[:], trf_d[ti * P : (ti + 1) * P, :])
                scr2 = stream.tile([P, D], bf16, tag="trscr")
                y2nt = res.tile([P, 1], f32, tag=f"y2n_{ti}")
                nc.vector.scalar_tensor_tensor(
                    scr2[:], trt[:], -GAMMA, trt[:], MUL, MUL, accum_out=y2nt[:]
                )

                for qc in range(NQC):
                    ps = sqpool.tile([P, QTILE], f32, tag="sq")
                    for dc in range(ND):
                        nc.tensor.matmul(
                            ps[:],
                            ttg[dc][:, tl * P : (tl + 1) * P],
                            xt_sb[dc][:, qc * QTILE : (qc + 1) * QTILE],
                            start=(dc == 0),
                            stop=(dc == ND - 1),
                        )
                    et = exppool.tile([P, QTILE], bf16, tag="exp")
                    nc.scalar.activation(
                        et[:], ps[:], AF.Exp, bias=y2nt[:], scale=2.0 * GAMMA
                    )
                    nc.tensor.matmul(
                        S_ps[qc][:],
                        coef_sb[:, ti : ti + 1],
                        et[:],
                        start=(ti == 0),
                        stop=(ti == NT - 1),
                    )

        # ---- epilogue: out = exp(-g*x2) * S, all in row layout ----
        ex2 = res.tile([1, QPC], f32, tag="ex2")
        nc.scalar.activation(ex2[:], x2row[:], AF.Exp, scale=-GAMMA)
        outrow = res.tile([1, QPC], f32, tag="outrow")
        for qc in range(NQC):
            nc.vector.tensor_mul(
                outrow[:, qc * QTILE : (qc + 1) * QTILE],
                ex2[:, qc * QTILE : (qc + 1) * QTILE],
                S_ps[qc][:],
            )
        nc.sync.dma_start(out_d.rearrange("(a q) -> a q", a=1), outrow[:])

    nc.compile()
    return nc


def _get_program():
    if "nc" not in _CACHE:
        _CACHE["nc"] = _build_program()
    return _CACHE["nc"]


def make_in_maps(X, train_X, dual_coef):
    bf = ml_dtypes.bfloat16
    ttb = np.ascontiguousarray(train_X.T).astype(bf)
    coefb = np.ascontiguousarray(dual_coef.reshape(NT, P).T).astype(bf)
    XT = np.ascontiguousarray(X.T)
    in_maps = []
    for c in range(N_CORES):
        in_maps.append(
            {
                "tt_bf16": ttb,
                "xt_bf16": np.ascontiguousarray(XT[:, c * QPC : (c + 1) * QPC]).astype(
                    bf
                ),
                "train_f32": train_X,
                "x_f32": np.ascontiguousarray(X[c * QPC : (c + 1) * QPC]),
                "coef_bf16": coefb,
            }
        )
    return in_maps


def _get_callable():
    """Cached (fn, in_names, out_names, out_avals, zero_outs, mesh) for the
    sharded 8-core NEFF execution."""
    if "call" in _CACHE:
        return _CACHE["call"]

    import jax
    from jax.sharding import Mesh, PartitionSpec
    from jax.experimental.shard_map import shard_map

    import concourse.mybir as mybir
    from concourse import bass2jax
    from concourse.bass2jax import install_neuronx_cc_hook

    install_neuronx_cc_hook()
    nc = _get_program()

    partition_name = (
        nc.partition_id_tensor.name if nc.partition_id_tensor else None
    )
    in_names, out_names, out_avals, zero_outs = [], [], [], []
    for alloc in nc.m.functions[0].allocations:
        if not isinstance(alloc, mybir.MemoryLocationSet):
            continue
        if alloc.kind not in ("ExternalInput", "ExternalOutput"):
            continue
        name = alloc.memorylocations[0].name
        if alloc.kind == "ExternalInput":
            if name != partition_name:
                in_names.append(name)
        else:
            out_names.append(name)
            shape = tuple(alloc.tensor_shape)
            dtype = mybir.dt.np(alloc.dtype)
            out_avals.append(jax.core.ShapedArray(shape, dtype))
            zero_outs.append(np.zeros(shape, dtype))
    all_in_names = in_names + out_names
    if partition_name is not None:
        all_in_names = all_in_names + [partition_name]

    def _body(*args):
        operands = list(args)
        if partition_name is not None:
            operands.append(bass2jax.partition_id_tensor())
        outs = bass2jax._bass_exec_p.bind(
            *operands,
            out_avals=tuple(out_avals),
            in_names=tuple(all_in_names),
            out_names=tuple(out_names),
            lowering_input_output_aliases=(),
            sim_require_finite=True,
            sim_require_nnan=True,
            nc=nc,
        )
        return tuple(outs)

    devices = jax.devices()[:N_CORES]
    mesh = Mesh(np.asarray(devices), ("core",))
    n_all = len(in_names) + len(out_names)
    fn = jax.jit(
        shard_map(
            _body,
            mesh=mesh,
            in_specs=(PartitionSpec("core"),) * n_all,
            out_specs=(PartitionSpec("core"),) * len(out_names),
            check_rep=False,
        ),
        keep_unused=True,
    )
    _CACHE["call"] = (fn, in_names, out_names, out_avals, zero_outs, mesh)
    return _CACHE["call"]


def concat_inputs(in_maps):
    fn, in_names, out_names, out_avals, zero_outs, mesh = _get_callable()
    concat_in = [
        np.concatenate([np.asarray(m[name]) for m in in_maps], axis=0)
        for name in in_names
    ]
    concat_zeros = [
        np.zeros((N_CORES * z.shape[0], *z.shape[1:]), z.dtype) for z in zero_outs
    ]
    return concat_in + concat_zeros


def kernel(X, train_X, dual_coef):
    X = np.asarray(X, dtype=np.float32)
    train_X = np.asarray(train_X, dtype=np.float32)
    dual_coef = np.asarray(dual_coef, dtype=np.float32)

    fn, in_names, out_names, out_avals, zero_outs, mesh = _get_callable()
    in_maps = make_in_maps(X, train_X, dual_coef)
    args = concat_inputs(in_maps)
    outs = fn(*args)
    out = np.asarray(outs[0]).reshape(-1)
    return out.astype(np.float32)



# revision 2
# speedup vs baseline: 1.9322x; 1.9322x over previous
"""RBF kernel ridge regression inference on 8 Trainium2 NeuronCores.

out[q] = sum_t exp(-gamma * ||X[q] - T[t]||^2) * coef[t]

Factored as sum_t (coef[t] * exp(-g*y2[t])) * exp(2g*dot[q,t] - g*x2[q]):
the train-side norm folds into the coefficients on the host, the query-side
norm folds into the ScalarE activation bias.  On device each core runs only
three pipelined stages per (query-chunk, train-group) tile:

  TensorE : fp8e4 DoubleRow GEMM (queries stationary)  -> psum [128q, 2048t]
  ScalarE : one wide Exp over 4 psum banks, bias=-g*x2 -> et bf16 [128, 2048]
  VectorE : et * coef' multiply-accumulate along free  -> partial out [128,1]

Queries are sharded across the 8 cores; train data and coefficients are
replicated.
"""

import numpy as np
import ml_dtypes

GAMMA = 1.0
N_QUERY, N_TRAIN, D = 8192, 8192, 512
N_CORES = 8
P = 128
QPC = N_QUERY // N_CORES   # 1024 queries per core
KD = D // P                # 4 contraction subtiles of 128
NQC = QPC // P             # 8 query chunks of 128 (stationary side)
G = 2048                   # train columns per psum group (4 banks)
NG = N_TRAIN // G          # 4 train groups
TT = 512                   # one psum bank / one matmul free dim

_CACHE = {}


def _build_program():
    from contextlib import ExitStack

    import concourse.bass as bass
    import concourse.mybir as mybir
    import concourse.tile as tile
    from concourse import bacc

    f32 = mybir.dt.float32
    bf16 = mybir.dt.bfloat16
    fp8 = mybir.dt.float8e4
    AF = mybir.ActivationFunctionType
    MUL = mybir.AluOpType.mult
    ADD = mybir.AluOpType.add
    DR = mybir.MatmulPerfMode.DoubleRow

    nc = bacc.Bacc(
        "TRN2", target_bir_lowering=False, debug=False, num_devices=N_CORES
    )

    tt_d = nc.dram_tensor("tt_fp8", [P, KD, N_TRAIN], fp8, kind="ExternalInput").ap()
    xt_d = nc.dram_tensor("xt_fp8", [P, KD, QPC], fp8, kind="ExternalInput").ap()
    coef_d = nc.dram_tensor("coef_rep", [P, N_TRAIN], bf16, kind="ExternalInput").ap()
    x2_d = nc.dram_tensor("x2neg", [P, NQC], f32, kind="ExternalInput").ap()
    out_d = nc.dram_tensor("out", [QPC], f32, kind="ExternalOutput").ap()

    with tile.TileContext(nc) as tc, ExitStack() as ctx:
        res = ctx.enter_context(tc.tile_pool(name="res", bufs=1))
        etp = ctx.enter_context(tc.tile_pool(name="etp", bufs=3))
        scrp = ctx.enter_context(tc.tile_pool(name="scrp", bufs=2))
        psq = ctx.enter_context(tc.tile_pool(name="psq", bufs=2, space="PSUM"))

        # ---- prologue: resident loads, split per group so compute overlaps ----
        xt_sb = res.tile([P, KD, QPC], fp8, tag="xt")
        nc.sync.dma_start(xt_sb[:], xt_d[:])
        x2c = res.tile([P, NQC], f32, tag="x2c")
        nc.sync.dma_start(x2c[:], x2_d[:])
        tt_sb = res.tile([P, KD, N_TRAIN], fp8, tag="tt")
        coef_sb = res.tile([P, N_TRAIN], bf16, tag="coef")
        for g in range(NG):
            nc.sync.dma_start(
                tt_sb[:, :, g * G : (g + 1) * G], tt_d[:, :, g * G : (g + 1) * G]
            )
            nc.sync.dma_start(
                coef_sb[:, g * G : (g + 1) * G], coef_d[:, g * G : (g + 1) * G]
            )

        pacc = res.tile([P, NQC * NG], f32, tag="pacc")
        outc = res.tile([P, NQC], f32, tag="outc")

        # ---- main loop: 4 train groups x 8 query chunks ----
        for g in range(NG):
            for qc in range(NQC):
                ps = psq.tile([P, G], f32, tag="ps")
                for kd2 in range(KD // 2):
                    w = xt_sb[:, 2 * kd2 : 2 * kd2 + 2, qc * P : (qc + 1) * P]
                    for k in range(G // TT):
                        t0 = g * G + k * TT
                        nc.tensor.matmul(
                            ps[:, k * TT : (k + 1) * TT],
                            w,
                            tt_sb[:, 2 * kd2 : 2 * kd2 + 2, t0 : t0 + TT],
                            start=(kd2 == 0),
                            stop=(kd2 == KD // 2 - 1),
                            perf_mode=DR,
                        )
                et = etp.tile([P, G], bf16, tag="et")
                nc.scalar.activation(
                    et[:], ps[:], AF.Exp, bias=x2c[:, qc : qc + 1], scale=2.0 * GAMMA
                )
                scr = scrp.tile([P, G], bf16, tag="scr")
                nc.vector.scalar_tensor_tensor(
                    scr[:], et[:], 1.0, coef_sb[:, g * G : (g + 1) * G], MUL, MUL,
                    accum_out=pacc[:, qc * NG + g : qc * NG + g + 1],
                )

        # ---- epilogue: reduce the 4 group partials per query chunk ----
        for qc in range(NQC):
            nc.vector.tensor_reduce(
                outc[:, qc : qc + 1],
                pacc[:, qc * NG : (qc + 1) * NG],
                mybir.AxisListType.X,
                ADD,
            )
        nc.sync.dma_start(out_d.rearrange("(c p) -> p c", p=P), outc[:])

    nc.compile()
    return nc


def _get_program():
    if "nc" not in _CACHE:
        _CACHE["nc"] = _build_program()
    return _CACHE["nc"]


def make_in_maps(X, train_X, dual_coef):
    fp8 = ml_dtypes.float8_e4m3
    bf = ml_dtypes.bfloat16

    # train side: [d, t] -> [d%128, d//128, t], fp8
    ttb = np.ascontiguousarray(
        train_X.T.reshape(KD, P, N_TRAIN).transpose(1, 0, 2)
    ).astype(fp8)
    # fold exp(-g*||T_t||^2) into the coefficients, replicate across partitions
    y2 = np.sum(train_X.astype(np.float32) ** 2, axis=1)
    coef_f = (dual_coef.astype(np.float32) * np.exp(-GAMMA * y2)).astype(bf)
    coefb = np.ascontiguousarray(np.broadcast_to(coef_f[None, :], (P, N_TRAIN)))

    in_maps = []
    for c in range(N_CORES):
        Xc = X[c * QPC : (c + 1) * QPC]
        xtb = np.ascontiguousarray(
            Xc.T.reshape(KD, P, QPC).transpose(1, 0, 2)
        ).astype(fp8)
        x2 = np.sum(Xc.astype(np.float32) ** 2, axis=1)
        x2neg = np.ascontiguousarray((-GAMMA * x2).reshape(NQC, P).T)
        in_maps.append(
            {
                "tt_fp8": ttb,
                "xt_fp8": xtb,
                "coef_rep": coefb,
                "x2neg": x2neg,
            }
        )
    return in_maps


def _get_callable():
    """Cached (fn, in_names, out_names, out_avals, zero_outs, mesh) for the
    sharded 8-core NEFF execution."""
    if "call" in _CACHE:
        return _CACHE["call"]

    import jax
    from jax.sharding import Mesh, PartitionSpec
    from jax.experimental.shard_map import shard_map

    import concourse.mybir as mybir
    from concourse import bass2jax
    from concourse.bass2jax import install_neuronx_cc_hook

    install_neuronx_cc_hook()
    nc = _get_program()

    partition_name = (
        nc.partition_id_tensor.name if nc.partition_id_tensor else None
    )
    in_names, out_names, out_avals, zero_outs = [], [], [], []
    for alloc in nc.m.functions[0].allocations:
        if not isinstance(alloc, mybir.MemoryLocationSet):
            continue
        if alloc.kind not in ("ExternalInput", "ExternalOutput"):
            continue
        name = alloc.memorylocations[0].name
        if alloc.kind == "ExternalInput":
            if name != partition_name:
                in_names.append(name)
        else:
            out_names.append(name)
            shape = tuple(alloc.tensor_shape)
            dtype = mybir.dt.np(alloc.dtype)
            out_avals.append(jax.core.ShapedArray(shape, dtype))
            zero_outs.append(np.zeros(shape, dtype))
    all_in_names = in_names + out_names
    if partition_name is not None:
        all_in_names = all_in_names + [partition_name]

    def _body(*args):
        operands = list(args)
        if partition_name is not None:
            operands.append(bass2jax.partition_id_tensor())
        outs = bass2jax._bass_exec_p.bind(
            *operands,
            out_avals=tuple(out_avals),
            in_names=tuple(all_in_names),
            out_names=tuple(out_names),
            lowering_input_output_aliases=(),
            sim_require_finite=True,
            sim_require_nnan=True,
            nc=nc,
        )
        return tuple(outs)

    devices = jax.devices()[:N_CORES]
    mesh = Mesh(np.asarray(devices), ("core",))
    n_all = len(in_names) + len(out_names)
    fn = jax.jit(
        shard_map(
            _body,
            mesh=mesh,
            in_specs=(PartitionSpec("core"),) * n_all,
            out_specs=(PartitionSpec("core"),) * len(out_names),
            check_rep=False,
        ),
        keep_unused=True,
    )
    _CACHE["call"] = (fn, in_names, out_names, out_avals, zero_outs, mesh)
    return _CACHE["call"]


def concat_inputs(in_maps):
    fn, in_names, out_names, out_avals, zero_outs, mesh = _get_callable()
    concat_in = [
        np.concatenate([np.asarray(m[name]) for m in in_maps], axis=0)
        for name in in_names
    ]
    concat_zeros = [
        np.zeros((N_CORES * z.shape[0], *z.shape[1:]), z.dtype) for z in zero_outs
    ]
    return concat_in + concat_zeros


def kernel(X, train_X, dual_coef):
    X = np.asarray(X, dtype=np.float32)
    train_X = np.asarray(train_X, dtype=np.float32)
    dual_coef = np.asarray(dual_coef, dtype=np.float32)

    fn, in_names, out_names, out_avals, zero_outs, mesh = _get_callable()
    in_maps = make_in_maps(X, train_X, dual_coef)
    args = concat_inputs(in_maps)
    outs = fn(*args)
    out = np.asarray(outs[0]).reshape(-1)
    return out.astype(np.float32)
